# revision 8
# baseline (speedup 1.0000x reference)
"""Trainium2 Bass kernel: per-sample hypernetwork depthwise 3x3 conv.

Reference computation (per batch b):
    W_dw[b] = (z[b] @ W_lin.T).reshape(OUT_C, 1, 3, 3)
    y[b]    = depthwise_conv2d(x[b], W_dw[b], padding=1)

Sharding: data-parallel over batch across 8 NeuronCores (2 batches/core),
W_lin replicated. Each core computes its own W_dw on-device.

Per-core design (v2):
  - channels (256) -> 2 groups of 128 on SBUF partitions
  - image split into 32-row bands; each band loaded by a GPSIMD (SWDGE)
    casting DMA straight from f32 HBM into an fp16 SBUF tile (34 rows:
    1 halo row above/below, halo rows chained from the previous band's
    tile via small SBUF->SBUF copies; image-boundary rows memset to 0)
  - 9 conv taps split across three engines per 16-row PSUM group:
      * ACT: tap t3 (dy=1,dx=-1) full-width product written directly to
        PSUM (the accumulation-group initializer); its width-edge wrap
        garbage is subtracted by a tiny strided PE matmul
      * DVE: the 3 center-column taps (dx=0) as tensor_scalar products
        + tensor_tensor adds into an fp16 accumulator
      * PE: the remaining 5 taps as row-split [4,127] diag matmuls
        (start=False onto ACT's init), plus a fold matmul adding the
        DVE accumulator into PSUM, plus the wrap correction
  - ACT drains each PSUM group to an f32 out tile; SyncE issues the
    output DMAs (per 16-row group)
  - W_dw computed on-device by 18 small fp32 matmuls from a host-side
    re-layout of W_lin (pure permutation/transpose, no host math)
"""

import os
import sys

for _p in ("/opt/trn_rl_repo", "/root/.axon_site", "/root/.axon_site/_ro/trn_rl_repo",
           "/root/.axon_site/_ro/pypackages"):
    if os.path.isdir(_p) and _p not in sys.path:
        sys.path.append(_p)

import numpy as np

import concourse.bass as bass
import concourse.tile as tile
from concourse import bacc, mybir
from concourse import bass_utils
from concourse.alu_op_type import AluOpType

F32 = mybir.dt.float32
F16 = mybir.dt.float16

# problem constants (hardcoded per contract)
B, OUT_C, H, W = 16, 256, 128, 128
K, Z_DIM = 3, 64
N_CORES = 8
B_PER = B // N_CORES          # 2 batches per core
G = OUT_C // 128              # 2 channel groups of 128

ROWS_BAND = 32
ROWS_GROUP = 16
N_BANDS = H // ROWS_BAND      # 4
GRP_PER_BAND = ROWS_BAND // ROWS_GROUP  # 2
TILE_ROWS = ROWS_BAND + 2     # 34 (halo above + below)
FLAT = TILE_ROWS * W          # 4352

# taps: t = dy*3 + dx, image offset (dy-1, dx-1)
ACT_TAP = 3                          # (dy=1, dx=-1): full width + corr
DVE_TAPS = (1, 4, 7)                 # center column (dx=0)
PE_TAPS = (0, 2, 5, 6, 8)            # row-split, edge-safe


def build_nc():
    nc = bacc.Bacc("TRN2", target_bir_lowering=False, debug=False)

    x_d = nc.dram_tensor("x", [B_PER, OUT_C, H, W], F32, kind="ExternalInput")
    zt_d = nc.dram_tensor("zT", [Z_DIM, B_PER], F32, kind="ExternalInput")
    wlt_d = nc.dram_tensor("wlt", [Z_DIM, OUT_C * K * K], F32, kind="ExternalInput")
    ident_d = nc.dram_tensor("ident", [128, 128], F32, kind="ExternalInput")
    y_d = nc.dram_tensor("y", [B_PER, OUT_C, H, W], F32, kind="ExternalOutput")

    n_chunks = OUT_C * K * K // 128          # 18
    wd_cols = K * K * G * B_PER              # 36, col = (g*9 + t)*b_per + b

    with tile.TileContext(nc) as tc:
        with tc.tile_pool(name="wconst", bufs=1) as wpool:
            # weight loads go on the SAME gpsimd queue as the band DMAs but
            # are emitted first, so their packets drain before the big band
            # transfers swamp the DMA engines (fp16 casts; precision ~5e-4)
            wlt16 = wpool.tile([Z_DIM, OUT_C * K * K], F16)
            third = OUT_C * K * K // 3
            for ci in range(3):
                nc.gpsimd.dma_start(wlt16[:, ci * third:(ci + 1) * third],
                                    wlt_d.ap()[:, ci * third:(ci + 1) * third])
            ztf = wpool.tile([Z_DIM, B_PER], F32)
            nc.sync.dma_start(ztf[:], zt_d.ap()[:, :])
            zt16 = wpool.tile([Z_DIM, B_PER], F16)
            nc.vector.tensor_scalar(out=zt16[:], in0=ztf[:], scalar1=1.0,
                                    scalar2=None, op0=AluOpType.mult)
            ident = wpool.tile([128, 128], F32)
            nc.sync.dma_start(ident[:], ident_d.ap()[:, :])
            actwarm = wpool.tile([128, 2], F32)
            nc.scalar.mul(actwarm[:], ident[:, 0:2], 1.0)

            wd = wpool.tile([128, wd_cols], F32)
            with tc.tile_pool(name="wpsum", bufs=4, space="PSUM") as wps:
                for j in range(n_chunks):
                    ps = wps.tile([128, B_PER], F32)
                    nc.tensor.matmul(ps[:], wlt16[:, 128 * j:128 * (j + 1)], zt16[:],
                                     start=True, stop=True)
                    nc.vector.tensor_copy(wd[:, B_PER * j:B_PER * (j + 1)], ps[:])

            def wcol(b, g, t):
                return (g * K * K + t) * B_PER + b

            # fp16 diag weights for the PE taps + negated ACT-tap weight
            identf16 = wpool.tile([128, 128], F16)
            nc.vector.tensor_scalar(out=identf16[:], in0=ident[:], scalar1=1.0,
                                    scalar2=None, op0=AluOpType.mult)
            diags = {}

            def make_diags(b, g):
                for t in PE_TAPS:
                    dt_ = wpool.tile([128, 128], F16, tag=f"d_{b}_{g}_{t}", name="dt_")
                    nc.vector.tensor_scalar(
                        out=dt_[:], in0=ident[:], scalar1=wd[:, wcol(b, g, t):wcol(b, g, t) + 1],
                        scalar2=None, op0=AluOpType.mult)
                    diags[(b, g, t)] = dt_
                dn = wpool.tile([128, 128], F16, tag=f"dn_{b}_{g}", name="dn")
                nc.vector.tensor_scalar(
                    out=dn[:], in0=ident[:], scalar1=wd[:, wcol(b, g, ACT_TAP):wcol(b, g, ACT_TAP) + 1],
                    scalar2=None, op0=AluOpType.mult)
                nc.vector.tensor_scalar(
                    out=dn[:], in0=dn[:], scalar1=-1.0,
                    scalar2=None, op0=AluOpType.mult)
                diags[(b, g, "neg")] = dn

            with tc.tile_pool(name="xband", bufs=8) as xpool, \
                 tc.tile_pool(name="oband", bufs=4) as opool, \
                 tc.tile_pool(name="accp", bufs=3) as accpool, \
                 tc.tile_pool(name="scrp", bufs=2) as scrpool, \
                 tc.tile_pool(name="psum", bufs=2, space="PSUM") as pspool:

                # PSUM has_written warmup: only TensorE matmuls set the
                # per-element has_written bit; a start=False matmul
                # OVERWRITES where the bit is clear. The main loop relies on
                # ACT writing the psum init with PE accumulating on top
                # (start=False), which only works once every element of both
                # psum bufs has been matmul-written. Do that once up front.
                dummy = wpool.tile([128, 512], F16)
                nc.vector.memset(dummy[:], 0.0)
                for _ in range(2):
                    wt_ = pspool.tile([128, ROWS_GROUP * W], F32, name="ps")
                    for k in range(4):
                        nc.tensor.matmul(wt_[:, 512 * k:512 * (k + 1)],
                                         identf16[:], dummy[:],
                                         start=True, stop=True)

                # pending ACT drain + output DMA, emitted one group late so
                # ACT's psum-init for group g+1 precedes the drain of group g
                pending = []

                def flush_pending():
                    while pending:
                        ot_, ps_, dma_args = pending.pop(0)
                        nc.scalar.copy(ot_[:], ps_[:])
                        nc.sync.dma_start(dma_args[0], ot_[:])

                for b in range(B_PER):
                    for g in range(G):
                        make_diags(b, g)
                        prev_xb = None
                        for band in range(N_BANDS):
                            r0 = band * ROWS_BAND
                            xb = xpool.tile([128, FLAT], F16)
                            xbv = xb[:].rearrange("p (t c) -> p t c", c=W)
                            # halo rows (tile rows 0,1 = image rows r0-1, r0)
                            if band == 0:
                                nc.vector.memset(xb[:, 0:W], 0.0)
                                lo, dst0 = 0, W
                            else:
                                nc.sync.dma_start(xb[:, 0:2 * W],
                                                  prev_xb[:, ROWS_BAND * W:(ROWS_BAND + 2) * W])
                                lo, dst0 = r0 + 1, 2 * W
                            hi = min(r0 + ROWS_BAND + 1, H)
                            if band == N_BANDS - 1:
                                nc.vector.memset(xb[:, (TILE_ROWS - 1) * W:], 0.0)
                            # main band load: f32 HBM -> fp16 SBUF casting DMA
                            if b == 0 and g == 0 and band == 0:
                                nc.gpsimd.dma_start(
                                    xb[:, W:19 * W],
                                    x_d.ap()[b, 0:128, 0:18, :])
                                nc.gpsimd.dma_start(
                                    xb[:, 19 * W:34 * W],
                                    x_d.ap()[b, 0:128, 18:33, :])
                            else:
                                nc.gpsimd.dma_start(
                                    xb[:, dst0:dst0 + (hi - lo) * W],
                                    x_d.ap()[b, 128 * g:128 * (g + 1), lo:hi, :])
                            prev_xb = xb[:]

                            for grp in range(GRP_PER_BAND):
                                g0 = grp * ROWS_GROUP      # tile row of dy=0 tap
                                ps = pspool.tile([128, ROWS_GROUP * W], F32, name="ps")
                                psv = ps[:].rearrange("p (t c) -> p t c", c=W)

                                # ---- ACT: tap t3 product -> psum (init) ----
                                # reads xb[(g0+1)*W - 1 : +2048]
                                nc.scalar.mul(ps[:],
                                              xb[:, (g0 + 1) * W - 1:(g0 + 1) * W - 1 + ROWS_GROUP * W],
                                              wd[:, wcol(b, g, ACT_TAP):wcol(b, g, ACT_TAP) + 1])

                                # ---- emit the previous group's drain+DMA ----
                                flush_pending()

                                # ---- DVE: 3 center taps -> fp16 acc ----
                                acc = accpool.tile([128, ROWS_GROUP * W], F16)
                                scr = scrpool.tile([128, ROWS_GROUP * W], F16)
                                t0, t1_, t2_ = DVE_TAPS
                                nc.vector.tensor_scalar(
                                    out=acc[:], in0=xb[:, g0 * W:g0 * W + ROWS_GROUP * W],
                                    scalar1=wd[:, wcol(b, g, t0):wcol(b, g, t0) + 1],
                                    scalar2=None, op0=AluOpType.mult)
                                nc.vector.tensor_scalar(
                                    out=scr[:], in0=xb[:, (g0 + 1) * W:(g0 + 1) * W + ROWS_GROUP * W],
                                    scalar1=wd[:, wcol(b, g, t1_):wcol(b, g, t1_) + 1],
                                    scalar2=None, op0=AluOpType.mult)
                                nc.vector.tensor_tensor(out=acc[:], in0=acc[:], in1=scr[:],
                                                        op=AluOpType.add)
                                nc.vector.tensor_scalar(
                                    out=scr[:], in0=xb[:, (g0 + 2) * W:(g0 + 2) * W + ROWS_GROUP * W],
                                    scalar1=wd[:, wcol(b, g, t2_):wcol(b, g, t2_) + 1],
                                    scalar2=None, op0=AluOpType.mult)
                                nc.vector.tensor_tensor(out=acc[:], in0=acc[:], in1=scr[:],
                                                        op=AluOpType.add)

                                # ---- PE: 5 row-split taps, fold acc, corr ----
                                for t in PE_TAPS:
                                    dy, dx = t // 3, t % 3
                                    for k in range(4):
                                        rr = g0 + dy + 4 * k
                                        if dx == 0:
                                            nc.tensor.matmul(
                                                psv[:, 4 * k:4 * k + 4, 1:128],
                                                diags[(b, g, t)][:],
                                                xbv[:, rr:rr + 4, 0:127],
                                                start=False, stop=False,
                                                skip_group_check=True)
                                        else:
                                            nc.tensor.matmul(
                                                psv[:, 4 * k:4 * k + 4, 0:127],
                                                diags[(b, g, t)][:],
                                                xbv[:, rr:rr + 4, 1:128],
                                                start=False, stop=False,
                                                skip_group_check=True)
                                for k in range(4):
                                    nc.tensor.matmul(
                                        ps[:, 512 * k:512 * (k + 1)], identf16[:],
                                        acc[:, 512 * k:512 * (k + 1)],
                                        start=False, stop=False, skip_group_check=True)
                                # corr: psum[:, j, 0] -= w3 * xb[g0+j, 127]
                                nc.tensor.matmul(
                                    psv[:, 0:ROWS_GROUP, 0:1],
                                    diags[(b, g, "neg")][:],
                                    xbv[:, g0:g0 + ROWS_GROUP, 127:128],
                                    start=False, stop=True, skip_group_check=True)

                                # ---- queue drain + out DMA (next iter) ----
                                ot = opool.tile([128, ROWS_GROUP * W], F32)
                                r_out = r0 + grp * ROWS_GROUP
                                pending.append(
                                    (ot, ps,
                                     (y_d.ap()[b, 128 * g:128 * (g + 1),
                                               r_out:r_out + ROWS_GROUP, :],)))
                flush_pending()

    nc.compile()
    return nc


def make_in_maps(x, z, W_lin, b_per=B_PER):
    """Host-side shard + layout transforms (no math)."""
    wl = np.asarray(W_lin, dtype=np.float32)
    wlperm = (wl.reshape(G, 128, K * K, Z_DIM)
                .transpose(0, 2, 1, 3)
                .reshape(OUT_C * K * K, Z_DIM))
    wlt = np.ascontiguousarray(wlperm.T)                  # [64, 2304]
    ident = np.eye(128, dtype=np.float32)
    x = np.asarray(x, dtype=np.float32)
    z = np.asarray(z, dtype=np.float32)
    in_maps = []
    for c in range(N_CORES):
        sl = slice(c * b_per, (c + 1) * b_per)
        in_maps.append({
            "x": np.ascontiguousarray(x[sl]),
            "zT": np.ascontiguousarray(z[sl].T),          # [64, b_per]
            "wlt": wlt,
            "ident": ident,
        })
    return in_maps


_NC_CACHE = {}


def kernel(x, z, W_lin):
    key = "main"
    if key not in _NC_CACHE:
        _NC_CACHE[key] = build_nc()
    nc = _NC_CACHE[key]
    in_maps = make_in_maps(x, z, W_lin)
    res = bass_utils.run_bass_kernel_spmd(nc, in_maps, core_ids=list(range(N_CORES)))
    out = np.concatenate([res.results[c]["y"] for c in range(N_CORES)], axis=0)
    return out.astype(np.float32, copy=False)


# revision 9
# speedup vs baseline: 1.0927x; 1.0927x over previous
"""Trainium2 Bass kernel: per-sample hypernetwork depthwise 3x3 conv.

Reference computation (per batch b):
    W_dw[b] = (z[b] @ W_lin.T).reshape(OUT_C, 1, 3, 3)
    y[b]    = depthwise_conv2d(x[b], W_dw[b], padding=1)

Sharding: data-parallel over batch across 8 NeuronCores (2 batches/core),
W_lin replicated. Each core computes its own W_dw on-device.

Per-core design (v2):
  - channels (256) -> 2 groups of 128 on SBUF partitions
  - image split into 32-row bands; each band loaded by a GPSIMD (SWDGE)
    casting DMA straight from f32 HBM into an fp16 SBUF tile (34 rows:
    1 halo row above/below, halo rows chained from the previous band's
    tile via small SBUF->SBUF copies; image-boundary rows memset to 0)
  - 9 conv taps split across three engines per 16-row PSUM group:
      * ACT: tap t3 (dy=1,dx=-1) full-width product written directly to
        PSUM (the accumulation-group initializer); its width-edge wrap
        garbage is subtracted by a tiny strided PE matmul
      * DVE: the 3 center-column taps (dx=0) as tensor_scalar products
        + tensor_tensor adds into an fp16 accumulator
      * PE: the remaining 5 taps as row-split [4,127] diag matmuls
        (start=False onto ACT's init), plus a fold matmul adding the
        DVE accumulator into PSUM, plus the wrap correction
  - ACT drains each PSUM group to an f32 out tile; SyncE issues the
    output DMAs (per 16-row group)
  - W_dw computed on-device by 18 small fp32 matmuls from a host-side
    re-layout of W_lin (pure permutation/transpose, no host math)
"""

import os
import sys

for _p in ("/opt/trn_rl_repo", "/root/.axon_site", "/root/.axon_site/_ro/trn_rl_repo",
           "/root/.axon_site/_ro/pypackages"):
    if os.path.isdir(_p) and _p not in sys.path:
        sys.path.append(_p)

import numpy as np

import concourse.bass as bass
import concourse.tile as tile
from concourse import bacc, mybir
from concourse import bass_utils
from concourse.alu_op_type import AluOpType

F32 = mybir.dt.float32
F16 = mybir.dt.float16

# problem constants (hardcoded per contract)
B, OUT_C, H, W = 16, 256, 128, 128
K, Z_DIM = 3, 64
N_CORES = 8
B_PER = B // N_CORES          # 2 batches per core
G = OUT_C // 128              # 2 channel groups of 128

ROWS_BAND = 32
ROWS_GROUP = 16
N_BANDS = H // ROWS_BAND      # 4
GRP_PER_BAND = ROWS_BAND // ROWS_GROUP  # 2
TILE_ROWS = ROWS_BAND + 2     # 34 (halo above + below)
FLAT = TILE_ROWS * W          # 4352

# taps: t = dy*3 + dx, image offset (dy-1, dx-1)
ACT_TAP = 3                          # (dy=1, dx=-1): full width + corr
DVE_TAPS = (1, 4, 7)                 # center column (dx=0)
PE_TAPS = (0, 2, 5, 6, 8)            # row-split, edge-safe


def build_nc():
    nc = bacc.Bacc("TRN2", target_bir_lowering=False, debug=False)

    x_d = nc.dram_tensor("x", [B_PER, OUT_C, H, W], F32, kind="ExternalInput")
    zt_d = nc.dram_tensor("zT", [Z_DIM, B_PER], F32, kind="ExternalInput")
    wlt_d = nc.dram_tensor("wlt", [Z_DIM, OUT_C * K * K], F32, kind="ExternalInput")
    ident_d = nc.dram_tensor("ident", [128, 128], F32, kind="ExternalInput")
    y_d = nc.dram_tensor("y", [B_PER, OUT_C, H, W], F32, kind="ExternalOutput")

    n_chunks = OUT_C * K * K // 128          # 18
    wd_cols = K * K * G * B_PER              # 36, col = (g*9 + t)*b_per + b

    with tile.TileContext(nc) as tc:
        with tc.tile_pool(name="wconst", bufs=1) as wpool:
            # weight loads go on the SAME gpsimd queue as the band DMAs but
            # are emitted first, so their packets drain before the big band
            # transfers swamp the DMA engines (fp16 casts; precision ~5e-4)
            wlt16 = wpool.tile([Z_DIM, OUT_C * K * K], F16)
            third = OUT_C * K * K // 3
            for ci in range(3):
                nc.gpsimd.dma_start(wlt16[:, ci * third:(ci + 1) * third],
                                    wlt_d.ap()[:, ci * third:(ci + 1) * third])
            ztf = wpool.tile([Z_DIM, B_PER], F32)
            nc.sync.dma_start(ztf[:], zt_d.ap()[:, :])
            zt16 = wpool.tile([Z_DIM, B_PER], F16)
            nc.vector.tensor_scalar(out=zt16[:], in0=ztf[:], scalar1=1.0,
                                    scalar2=None, op0=AluOpType.mult)
            ident = wpool.tile([128, 128], F32)
            nc.sync.dma_start(ident[:], ident_d.ap()[:, :])
            actwarm = wpool.tile([128, 2], F32)
            nc.scalar.mul(actwarm[:], ident[:, 0:2], 1.0)

            wd = wpool.tile([128, wd_cols], F32)
            with tc.tile_pool(name="wpsum", bufs=4, space="PSUM") as wps:
                for j in range(n_chunks):
                    ps = wps.tile([128, B_PER], F32)
                    nc.tensor.matmul(ps[:], wlt16[:, 128 * j:128 * (j + 1)], zt16[:],
                                     start=True, stop=True)
                    nc.vector.tensor_copy(wd[:, B_PER * j:B_PER * (j + 1)], ps[:])

            def wcol(b, g, t):
                return (g * K * K + t) * B_PER + b

            # fp16 diag weights for the PE taps + negated ACT-tap weight
            identf16 = wpool.tile([128, 128], F16)
            nc.vector.tensor_scalar(out=identf16[:], in0=ident[:], scalar1=1.0,
                                    scalar2=None, op0=AluOpType.mult)
            diags = {}

            def make_diags(b, g):
                for t in PE_TAPS:
                    dt_ = wpool.tile([128, 128], F16, tag=f"d_{b}_{g}_{t}", name="dt_")
                    nc.vector.tensor_scalar(
                        out=dt_[:], in0=ident[:], scalar1=wd[:, wcol(b, g, t):wcol(b, g, t) + 1],
                        scalar2=None, op0=AluOpType.mult)
                    diags[(b, g, t)] = dt_
                dn = wpool.tile([128, 128], F16, tag=f"dn_{b}_{g}", name="dn")
                nc.vector.tensor_scalar(
                    out=dn[:], in0=ident[:], scalar1=wd[:, wcol(b, g, ACT_TAP):wcol(b, g, ACT_TAP) + 1],
                    scalar2=None, op0=AluOpType.mult)
                nc.vector.tensor_scalar(
                    out=dn[:], in0=dn[:], scalar1=-1.0,
                    scalar2=None, op0=AluOpType.mult)
                diags[(b, g, "neg")] = dn

            with tc.tile_pool(name="xband", bufs=8) as xpool, \
                 tc.tile_pool(name="oband", bufs=4) as opool, \
                 tc.tile_pool(name="accp", bufs=3) as accpool, \
                 tc.tile_pool(name="scrp", bufs=2) as scrpool, \
                 tc.tile_pool(name="psum", bufs=2, space="PSUM") as pspool:

                # PSUM has_written warmup: only TensorE matmuls set the
                # per-element has_written bit; a start=False matmul
                # OVERWRITES where the bit is clear. The main loop relies on
                # ACT writing the psum init with PE accumulating on top
                # (start=False), which only works once every element of both
                # psum bufs has been matmul-written. Do that once up front.
                dummy = wpool.tile([128, 512], F16)
                nc.vector.memset(dummy[:], 0.0)
                zrow = dummy[:, 0:2 * W]
                for _ in range(2):
                    wt_ = pspool.tile([128, ROWS_GROUP * W], F32, name="ps")
                    for k in range(4):
                        nc.tensor.matmul(wt_[:, 512 * k:512 * (k + 1)],
                                         identf16[:], dummy[:],
                                         start=True, stop=True)

                # pending ACT drain + output DMA, emitted one group late so
                # ACT's psum-init for group g+1 precedes the drain of group g
                pending = []

                def flush_pending():
                    while pending:
                        ot_, ps_, dma_args = pending.pop(0)
                        nc.scalar.copy(ot_[:], ps_[:])
                        nc.sync.dma_start(dma_args[0], ot_[:])

                for b in range(B_PER):
                    for g in range(G):
                        make_diags(b, g)
                        prev_xb = None
                        for band in range(N_BANDS):
                            r0 = band * ROWS_BAND
                            xb = xpool.tile([128, FLAT], F16)
                            xbv = xb[:].rearrange("p (t c) -> p t c", c=W)
                            # halo rows (tile rows 0,1 = image rows r0-1, r0)
                            if band == 0:
                                nc.sync.dma_start(xb[:, 0:W], zrow[:, 0:W])
                                lo, dst0 = 0, W
                            else:
                                nc.sync.dma_start(xb[:, 0:2 * W],
                                                  prev_xb[:, ROWS_BAND * W:(ROWS_BAND + 2) * W])
                                lo, dst0 = r0 + 1, 2 * W
                            hi = min(r0 + ROWS_BAND + 1, H)
                            if band == N_BANDS - 1:
                                nc.sync.dma_start(xb[:, (TILE_ROWS - 1) * W:], zrow[:, 0:W])
                            # main band load: f32 HBM -> fp16 SBUF casting DMA
                            if b == 0 and g == 0 and band == 0:
                                nc.gpsimd.dma_start(
                                    xb[:, W:19 * W],
                                    x_d.ap()[b, 0:128, 0:18, :])
                                nc.gpsimd.dma_start(
                                    xb[:, 19 * W:34 * W],
                                    x_d.ap()[b, 0:128, 18:33, :])
                            else:
                                nc.gpsimd.dma_start(
                                    xb[:, dst0:dst0 + (hi - lo) * W],
                                    x_d.ap()[b, 128 * g:128 * (g + 1), lo:hi, :])
                            prev_xb = xb[:]

                            for grp in range(GRP_PER_BAND):
                                g0 = grp * ROWS_GROUP      # tile row of dy=0 tap
                                ps = pspool.tile([128, ROWS_GROUP * W], F32, name="ps")
                                psv = ps[:].rearrange("p (t c) -> p t c", c=W)

                                # ---- ACT: tap t3 product -> psum (init) ----
                                # reads xb[(g0+1)*W - 1 : +2048]
                                nc.scalar.mul(ps[:],
                                              xb[:, (g0 + 1) * W - 1:(g0 + 1) * W - 1 + ROWS_GROUP * W],
                                              wd[:, wcol(b, g, ACT_TAP):wcol(b, g, ACT_TAP) + 1])

                                # ---- emit the previous group's drain+DMA ----
                                flush_pending()

                                # ---- DVE: 3 center taps -> fp16 acc ----
                                acc = accpool.tile([128, ROWS_GROUP * W], F16)
                                scr = scrpool.tile([128, ROWS_GROUP * W], F16)
                                t0, t1_, t2_ = DVE_TAPS
                                nc.vector.tensor_scalar(
                                    out=acc[:], in0=xb[:, g0 * W:g0 * W + ROWS_GROUP * W],
                                    scalar1=wd[:, wcol(b, g, t0):wcol(b, g, t0) + 1],
                                    scalar2=None, op0=AluOpType.mult)
                                nc.vector.tensor_scalar(
                                    out=scr[:], in0=xb[:, (g0 + 1) * W:(g0 + 1) * W + ROWS_GROUP * W],
                                    scalar1=wd[:, wcol(b, g, t1_):wcol(b, g, t1_) + 1],
                                    scalar2=None, op0=AluOpType.mult)
                                nc.vector.tensor_tensor(out=acc[:], in0=acc[:], in1=scr[:],
                                                        op=AluOpType.add)
                                nc.vector.tensor_scalar(
                                    out=scr[:], in0=xb[:, (g0 + 2) * W:(g0 + 2) * W + ROWS_GROUP * W],
                                    scalar1=wd[:, wcol(b, g, t2_):wcol(b, g, t2_) + 1],
                                    scalar2=None, op0=AluOpType.mult)
                                nc.vector.tensor_tensor(out=acc[:], in0=acc[:], in1=scr[:],
                                                        op=AluOpType.add)

                                # ---- PE: 5 row-split taps, fold acc, corr ----
                                for t in PE_TAPS:
                                    dy, dx = t // 3, t % 3
                                    for k in range(4):
                                        rr = g0 + dy + 4 * k
                                        if dx == 0:
                                            nc.tensor.matmul(
                                                psv[:, 4 * k:4 * k + 4, 1:128],
                                                diags[(b, g, t)][:],
                                                xbv[:, rr:rr + 4, 0:127],
                                                start=False, stop=False,
                                                skip_group_check=True)
                                        else:
                                            nc.tensor.matmul(
                                                psv[:, 4 * k:4 * k + 4, 0:127],
                                                diags[(b, g, t)][:],
                                                xbv[:, rr:rr + 4, 1:128],
                                                start=False, stop=False,
                                                skip_group_check=True)
                                for k in range(4):
                                    nc.tensor.matmul(
                                        ps[:, 512 * k:512 * (k + 1)], identf16[:],
                                        acc[:, 512 * k:512 * (k + 1)],
                                        start=False, stop=False, skip_group_check=True)
                                # corr: psum[:, j, 0] -= w3 * xb[g0+j, 127]
                                nc.tensor.matmul(
                                    psv[:, 0:ROWS_GROUP, 0:1],
                                    diags[(b, g, "neg")][:],
                                    xbv[:, g0:g0 + ROWS_GROUP, 127:128],
                                    start=False, stop=True, skip_group_check=True)

                                # ---- queue drain + out DMA (next iter) ----
                                ot = opool.tile([128, ROWS_GROUP * W], F32)
                                r_out = r0 + grp * ROWS_GROUP
                                pending.append(
                                    (ot, ps,
                                     (y_d.ap()[b, 128 * g:128 * (g + 1),
                                               r_out:r_out + ROWS_GROUP, :],)))
                flush_pending()

    nc.compile()
    return nc


def make_in_maps(x, z, W_lin, b_per=B_PER):
    """Host-side shard + layout transforms (no math)."""
    wl = np.asarray(W_lin, dtype=np.float32)
    wlperm = (wl.reshape(G, 128, K * K, Z_DIM)
                .transpose(0, 2, 1, 3)
                .reshape(OUT_C * K * K, Z_DIM))
    wlt = np.ascontiguousarray(wlperm.T)                  # [64, 2304]
    ident = np.eye(128, dtype=np.float32)
    x = np.asarray(x, dtype=np.float32)
    z = np.asarray(z, dtype=np.float32)
    in_maps = []
    for c in range(N_CORES):
        sl = slice(c * b_per, (c + 1) * b_per)
        in_maps.append({
            "x": np.ascontiguousarray(x[sl]),
            "zT": np.ascontiguousarray(z[sl].T),          # [64, b_per]
            "wlt": wlt,
            "ident": ident,
        })
    return in_maps


_NC_CACHE = {}


def kernel(x, z, W_lin):
    key = "main"
    if key not in _NC_CACHE:
        _NC_CACHE[key] = build_nc()
    nc = _NC_CACHE[key]
    in_maps = make_in_maps(x, z, W_lin)
    res = bass_utils.run_bass_kernel_spmd(nc, in_maps, core_ids=list(range(N_CORES)))
    out = np.concatenate([res.results[c]["y"] for c in range(N_CORES)], axis=0)
    return out.astype(np.float32, copy=False)
